# revision 1
# baseline (speedup 1.0000x reference)
# Gaussian-smoothing heatmap kernel for trn2 (8 NeuronCores, data-parallel).
#
# Math: each heatmap channel is a single one-hot spike (or empty), so the
# 24->24 5x5 conv equals stamping the flipped 5x5 filter at each keypoint and
# summing over input channels.  All (o,i) filter slices are identical, so
# every output channel of a batch equals the same 64x64 accumulated map M[b].
# Per core: build row/col one-hot selectors (vector engine, bf16), contract
# 120 tap/keypoint rows against the banded filter matrix (tensor engine,
# bf16 in / fp32 accum), flatten each map into one SBUF partition via a
# DRAM roundtrip on the SWDGE queue (whose ring is separate from the two
# HWDGE output rings, so it overlaps the output stream), then stream
# out[b, o] for all 24 o with replicating DMAs whose contiguous runs are
# 16KB.
#
# Perf notes (from NTFF traces):
# - The 403MB fp32 output write dominates: ~50MB/core at ~400GB/s.
# - Group sizes ramp (8,8,16,32,32,32) so the first output DMA issues early.
# - Every group presents 32 outer entries to the DGE (it assigns descriptors
#   to the 16 SDMA engines by outer index) and reads all 32 stride-4
#   partitions (all 16 SBUF AXI ports): small groups replicate each map into
#   copies=32/GB partition blocks via log2(copies) SBUF->SBUF block-doubling
#   DMAs.  Entry layout is c-major: partition 4*(c*GB+b); entry (c,b) serves
#   channels [c*ch, (c+1)*ch), ch=24/copies, split half/half over the two
#   HWDGE queues.
# - Matmul operands are bf16 (one-hots exact; filter weights round to ~0.4%,
#   far under the 2e-2 gate) -- fp32 matmul runs LOW_HIGH double passes.
# - Host precomputes the [120,128] row/col target tables (coord rounding +
#   vis masking), packed with the filter matrix into one input DMA.
import numpy as np

B_FULL = 1024
K = 24
H = 64
N_CORES = 8
B_LOC = B_FULL // N_CORES  # 128
GROUPS = [8, 8, 16, 32, 32, 32]  # sum = B_LOC
SENT = 4096.0  # sentinel shift for masked-out keypoints

_CACHE = {}


def _build_nc():
    import concourse.mybir as mybir
    from concourse import bacc
    from concourse.tile import TileContext

    fp32 = mybir.dt.float32
    bf16 = mybir.dt.bfloat16
    i32 = mybir.dt.int32
    Alu = mybir.AluOpType

    nc = bacc.Bacc()
    # packed: [:, 0:128]=pyt, [:, 128:256]=pxt, [:, 256:376]=wg
    packed = nc.dram_tensor("packed", [120, 376], fp32, kind="ExternalInput")
    outT = nc.dram_tensor("out", [B_LOC, K, H * H], bf16, kind="ExternalOutput")

    with TileContext(nc) as tc:
        with (
            tc.tile_pool(name="const", bufs=1) as cpool,
            tc.tile_pool(name="big", bufs=2) as bpool,
            tc.tile_pool(name="stage", bufs=3) as stpool,
            tc.tile_pool(name="fbuf", bufs=5) as fpool,
            tc.tile_pool(name="dram", bufs=3, space="DRAM") as dpool,
            tc.tile_pool(name="ps_b", bufs=2, space="PSUM") as ps_b,
            tc.tile_pool(name="ps_map", bufs=3, space="PSUM") as ps_map,
        ):
            pk = cpool.tile([120, 376], fp32)
            nc.sync.dma_start(pk, packed[:, :])
            PYT = pk[:, 0:128]
            PXT = pk[:, 128:256]
            wgb = cpool.tile([120, 120], bf16)
            nc.vector.tensor_copy(wgb, pk[:, 256:376])

            io64i = cpool.tile([120, H], i32)
            nc.gpsimd.iota(io64i, pattern=[[1, H]], base=0, channel_multiplier=0)
            io64f = cpool.tile([120, H], fp32)
            nc.vector.tensor_copy(io64f, io64i)

            b0 = 0
            for GB in GROUPS:
                copies = 32 // GB
                ch = K // copies  # channels per copy block
                hch = ch // 2  # per queue
                rowsel = bpool.tile([120, GB * H], bf16, tag="rowsel")
                nc.vector.tensor_tensor(
                    rowsel.rearrange("p (b y) -> p b y", y=H),
                    io64f.unsqueeze(1).broadcast_to([120, GB, H]),
                    PYT[:, b0 : b0 + GB].unsqueeze(2).broadcast_to([120, GB, H]),
                    Alu.is_equal,
                )
                ohc = bpool.tile([120, GB * H], bf16, tag="ohc")
                nc.vector.tensor_tensor(
                    ohc.rearrange("p (b x) -> p b x", x=H),
                    io64f.unsqueeze(1).broadcast_to([120, GB, H]),
                    PXT[:, b0 : b0 + GB].unsqueeze(2).broadcast_to([120, GB, H]),
                    Alu.is_equal,
                )
                bbig = bpool.tile([120, GB * H], bf16, tag="bbig")
                for j in range(GB * H // 512):
                    psb = ps_b.tile([120, 512], fp32, tag="psb")
                    nc.tensor.matmul(
                        psb, lhsT=wgb, rhs=ohc[:, j * 512 : (j + 1) * 512],
                        start=True, stop=True,
                    )
                    # PSUM->SBUF copies run on the ACT engine, keeping the
                    # vector engine free for the next group's selectors
                    nc.scalar.copy(bbig[:, j * 512 : (j + 1) * 512], psb)

                # flatten via DRAM roundtrip on SWDGE (its ring is separate
                # from the two HWDGE output rings, so it overlaps the output
                # stream), chunked per 8 batches so the gather pipeline runs
                # while later psm chunks still compute.  Each batch map lands
                # contiguous (16KB) in one partition: b-major entry layout,
                # entry e=(b,c) at partition 4*(b*copies+c); gather fills the
                # c=0 slots.
                sg = stpool.tile([H, GB * H], bf16, tag="sg")
                d1 = dpool.tile([H, GB * H], bf16, tag="d1")
                Fg = fpool.tile([128, H * H], bf16, tag="F")
                for w in range(GB // 8):
                    psm = ps_map.tile([H, 512], fp32, tag="psm")
                    for s in range(8):
                        bl = w * 8 + s
                        nc.tensor.matmul(
                            psm[:, s * H : (s + 1) * H],
                            lhsT=rowsel[:, bl * H : (bl + 1) * H],
                            rhs=bbig[:, bl * H : (bl + 1) * H],
                            start=True,
                            stop=True,
                        )
                    cw = slice(w * 512, (w + 1) * 512)
                    nc.scalar.copy(sg[:, cw], psm)
                    if GB == 8:
                        # head groups: the HWDGE rings are still near-empty,
                        # so flatten with direct per-batch SBUF->SBUF DMAs
                        # (skips the DRAM roundtrip and its two HBM receipts).
                        # Steady-state groups must NOT do this: thousands of
                        # 256B descriptors inside the HWDGE rings stretch the
                        # 16KB output packets ~2x.
                        for j in range(8):
                            eng = nc.sync if j % 2 == 0 else nc.scalar
                            p = 4 * j * copies
                            eng.dma_start(
                                Fg[p : p + 1, :].rearrange("p (y x) -> p y x", x=H),
                                sg[:, j * H : (j + 1) * H].unsqueeze(1),
                            )
                    else:
                        nc.gpsimd.dma_start(d1[:, cw], sg[:, cw])
                        nc.gpsimd.dma_start(
                            Fg[32 * w * copies : 32 * (w + 1) * copies : 4 * copies, :]
                            .rearrange("b (y x) -> b y x", x=H),
                            d1[:, cw].rearrange("y (b x) -> b y x", x=H),
                        )
                # replicate to copy slots c'=1.. via SBUF->SBUF copies, one
                # DMA per c' covering all GB maps (16KB descriptors, GB outer
                # entries, no HBM traffic, no serial chains -- all read c=0)
                for cp in range(1, copies):
                    eng = nc.sync if cp % 2 == 1 else nc.scalar
                    eng.dma_start(
                        Fg[4 * cp : 128 : 4 * copies, :],
                        Fg[0 : 128 : 4 * copies, :],
                    )

                # replicated output write: 32 outer entries (b-major), entry
                # (b,c) covers channels [c*ch, (c+1)*ch), halves per queue
                src = Fg[0:128:4, :].unsqueeze(1)
                dst = outT[b0 : b0 + GB].rearrange(
                    "b (c q) n -> (b c) (q n)", c=copies
                )  # [32, ch*H*H], each row one (b,c) entry's copy block
                hr = hch * H * H
                nc.sync.dma_start(
                    dst[:, 0:hr], src.broadcast_to([32, hch, H * H])
                )
                nc.scalar.dma_start(
                    dst[:, hr : 2 * hr], src.broadcast_to([32, hch, H * H])
                )
                b0 += GB

    nc.compile()
    return nc


def _get_nc():
    if "nc" not in _CACHE:
        _CACHE["nc"] = _build_nc()
    return _CACHE["nc"]


def _host_inputs(x, weight, vis_batch, vis_kps):
    f1 = np.float32
    # coords: round(((x+1)*0.5)*63) in fp32, RNE -- bit-exact with jnp.round
    c = np.round((x.astype(f1) + f1(1.0)) * f1(0.5) * f1(63.0)).astype(np.int32)
    invalid = np.any((c >= H) | (c < 0), axis=-1)  # [B, K]
    c = np.where(invalid[..., None], 0, c)
    cx, cy = c[..., 0], c[..., 1]
    place = cx != 0  # torch quirk: only stamps where x-coord nonzero
    kill = np.zeros((B_FULL, K), bool)
    kill[vis_batch.astype(np.int64), vis_kps.astype(np.int64)] = True
    mask = place & ~kill  # [B, K]

    # pyt[u*24+k, b] = cy + u - 2 + SENT*(1-mask); pxt[c*24+k, b] = cx + c - 2
    # (cx/cy already zeroed for invalid rows; the row-side sentinel alone
    # suppresses masked stamps since rowsel becomes all-zero)
    u = np.arange(5, dtype=f1)[:, None, None]  # [5,1,1]
    pyt_all = cy.T[None].astype(f1) + u - f1(2.0) + f1(SENT) * (~mask).T[None]
    pxt_all = cx.T[None].astype(f1) + u - f1(2.0)
    pyt_all = pyt_all.reshape(5 * K, B_FULL)  # [(u,k), b]
    pxt_all = pxt_all.reshape(5 * K, B_FULL)

    gflip = np.ascontiguousarray(weight[0, 0][::-1, ::-1]).astype(f1)
    wgm = np.zeros((120, 120), f1)
    idx = np.arange(K)
    for uu in range(5):
        for cc in range(5):
            wgm[cc * K + idx, uu * K + idx] = gflip[uu, cc]

    in_maps = []
    for core in range(N_CORES):
        sl = slice(core * B_LOC, (core + 1) * B_LOC)
        packed = np.empty((120, 376), f1)
        packed[:, 0:128] = pyt_all[:, sl]
        packed[:, 128:256] = pxt_all[:, sl]
        packed[:, 256:376] = wgm
        in_maps.append({"packed": np.ascontiguousarray(packed)})
    return in_maps


def kernel(x, weight, vis_batch, vis_kps, _trace=False, _tmpdir=None):
    from concourse.bass_utils import run_bass_kernel_spmd

    nc = _get_nc()
    in_maps = _host_inputs(
        np.asarray(x), np.asarray(weight), np.asarray(vis_batch), np.asarray(vis_kps)
    )
    res = run_bass_kernel_spmd(
        nc, in_maps, core_ids=list(range(N_CORES)), trace=_trace, tmpdir=_tmpdir
    )
    out = np.concatenate(
        [r["out"].astype(np.float32).reshape(B_LOC, K, H, H) for r in res.results],
        axis=0,
    )
    if _trace:
        kernel._last_results = res
    return out

